# revision 1
# baseline (speedup 1.0000x reference)
"""Trainium2 Bass kernel for nn_Attention_46420006535531.

Gated multi-head attention with additive attention bias:
    q = x@Wq, (k, v) = split(x@Wkv), heads=8, dim_head=64
    attn = softmax(q*k^T*scale + bias); out = attn@v
    out = (out * sigmoid(x@Wg + bg)) @ Wo + bo

Sharding: 8 cores; core c handles batch b=c//2 and the 4 heads
4*(c%2)..4*(c%2)+3 (tensor-parallel over heads within a batch pair).
Each core computes a partial y (its heads' slice of Wo rows); the host
sums the two partials per batch and adds bo.

On-core layout: everything transposed.  S^T[j,i] = k@q^T + bias^T is
computed per head in [j, i] tiles; bias^T enters PSUM via an
identity-matmul accumulate (start=True) so no elementwise add pass is
needed; ACT does exp straight out of PSUM.  The softmax denominators
come for free from a ones-column prepended to each head's v block
(row 0 of the AV output).  All matmuls run in fp32r (full PE rate for
free dims >= 256, ~1e-4 relative error).
"""
import sys
import numpy as np

for _p in ("/opt/trn_rl_repo",):
    if _p not in sys.path:
        sys.path.insert(0, _p)

import concourse.bass as bass
import concourse.bacc as bacc
import concourse.tile as tile
from concourse import mybir
from concourse.bass_utils import run_bass_kernel_spmd

B, N, DIM = 4, 1024, 256
HEADS, DIM_HEAD, INNER = 8, 64, 512
HPC = 4                      # heads per core
NCORES = 8
SCALE = DIM_HEAD ** -0.5     # folded into Wq on the host

F32 = mybir.dt.float32
F32R = mybir.dt.float32r
BF16 = mybir.dt.bfloat16
FP16 = mybir.dt.float16
AF = mybir.ActivationFunctionType

NB = N // 512                # 2 i-blocks of 512
NJP = N // 128               # 8 j partition tiles
KK = DIM // 128              # 2 k-tiles for the projections


def _build_program(reps=1, loop_iters=0, ablate=()):
    nc = bacc.Bacc(None, target_bir_lowering=False)

    # ---- DRAM I/O (per core) ----
    xt_d = nc.dram_tensor("xt", [128, KK, N], F32, kind="ExternalInput")
    bias_d = nc.dram_tensor("bias_t", [HPC, NB, 128, NJP * 512], FP16,
                            kind="ExternalInput")
    wq_d = nc.dram_tensor("wq", [128, KK, 256], F32, kind="ExternalInput")
    wk_d = nc.dram_tensor("wk", [128, KK, 256], F32, kind="ExternalInput")
    wv_d = nc.dram_tensor("wv", [128, KK, 256], F32, kind="ExternalInput")
    wg_d = nc.dram_tensor("wg", [128, KK, HPC * 65], F32, kind="ExternalInput")
    bg_d = nc.dram_tensor("bg", [65, HPC], F32, kind="ExternalInput")
    wo_d = nc.dram_tensor("wo", [HPC, 65, 256], F32, kind="ExternalInput")
    on_d = nc.dram_tensor("ones65", [1, 65], F32, kind="ExternalInput")
    onc_d = nc.dram_tensor("ones128", [128, HPC], F32, kind="ExternalInput")
    y_d = nc.dram_tensor("y", [N, 256], F32, kind="ExternalOutput")

    with tile.TileContext(nc) as tc:
        import contextlib
        with contextlib.ExitStack() as ctx:
            const = ctx.enter_context(tc.tile_pool(name="const", bufs=1))
            acts = ctx.enter_context(tc.tile_pool(name="acts", bufs=1))
            biasp = ctx.enter_context(tc.tile_pool(name="biasp", bufs=3))
            pexp = ctx.enter_context(tc.tile_pool(name="pexp", bufs=4))
            pmul = ctx.enter_context(tc.tile_pool(name="pmul", bufs=10))
            small = ctx.enter_context(tc.tile_pool(name="small", bufs=4))
            outp = ctx.enter_context(tc.tile_pool(name="outp", bufs=8))
            gatep = ctx.enter_context(tc.tile_pool(name="gatep", bufs=8))
            ps_big = ctx.enter_context(tc.tile_pool(name="ps_big", bufs=4, space="PSUM"))
            ps_o = ctx.enter_context(tc.tile_pool(name="ps_o", bufs=2, space="PSUM"))
            ps_m = ctx.enter_context(tc.tile_pool(name="ps_m", bufs=2, space="PSUM"))

            # ---- constants / weights into SBUF ----
            ones65 = const.tile([1, 65], F32R, tag="ones65")
            nc.sync.dma_start(out=ones65[:], in_=on_d[:].bitcast(F32R))
            bg_sb = const.tile([65, HPC], F32, tag="bg")
            nc.sync.dma_start(out=bg_sb[:], in_=bg_d[:])
            wq_sb = const.tile([128, KK, 256], F32R, tag="wq")
            nc.sync.dma_start(out=wq_sb[:], in_=wq_d[:].bitcast(F32R))
            wk_sb = const.tile([128, KK, 256], F32R, tag="wk")
            nc.sync.dma_start(out=wk_sb[:], in_=wk_d[:].bitcast(F32R))
            wv_sb = const.tile([128, KK, 256], F32R, tag="wv")
            nc.sync.dma_start(out=wv_sb[:], in_=wv_d[:].bitcast(F32R))
            wg_sb = const.tile([128, KK, HPC * 65], F32R, tag="wg")
            nc.sync.dma_start(out=wg_sb[:], in_=wg_d[:].bitcast(F32R))
            wo_sb = []
            for h in range(HPC):
                t = const.tile([65, 256], F32R, tag=f"wo{h}")
                nc.sync.dma_start(out=t[:], in_=wo_d[h].bitcast(F32R))
                wo_sb.append(t)
            xt_sb = const.tile([128, KK, N], F32R, tag="xt")
            nc.sync.dma_start(out=xt_sb[:], in_=xt_d[:].bitcast(F32R))

            lp = nc.allow_low_precision(reason="fp32r attention pipeline")
            lp.__enter__()

            if loop_iters:
                with tc.For_i(0, loop_iters, 1):
                    _emit_body(nc, tc, locals(), ablate)
            else:
                for _rep in range(reps):
                    _emit_body(nc, tc, locals(), ablate)

            lp.__exit__(None, None, None)

    nc.compile()
    return nc


def _emit_body(nc, tc, env, ablate=()):
    const = env["const"]; acts = env["acts"]; biasp = env["biasp"]
    pexp = env["pexp"]; pmul = env["pmul"]; small = env["small"]; outp = env["outp"]
    gatep = env["gatep"]; ps_big = env["ps_big"]; ps_o = env["ps_o"]
    ps_m = env["ps_m"]
    ones65 = env["ones65"]; bg_sb = env["bg_sb"]
    wq_sb = env["wq_sb"]; wk_sb = env["wk_sb"]; wv_sb = env["wv_sb"]
    wg_sb = env["wg_sb"]; wo_sb = env["wo_sb"]; xt_sb = env["xt_sb"]
    bias_d = env["bias_d"]; onc_d = env["onc_d"]; y_d = env["y_d"]
    if True:
            # ---- phase 1: projections ----
            # qT / kT as two head-pair tiles [128, N] (heads 2p, 2p+1)
            qT, kT = [], []
            for p in range(2):
                qt = acts.tile([128, N], F32R, tag=f"qT{p}")
                kt = acts.tile([128, N], F32R, tag=f"kT{p}")
                qT.append(qt)
                kT.append(kt)
                for ib in range(NB):
                    psq = ps_big.tile([128, 512], F32, tag="big")
                    psk = ps_big.tile([128, 512], F32, tag="big")
                    for kk in range(KK):
                        nc.tensor.matmul(
                            psq[:], lhsT=wq_sb[:, kk, 128 * p:128 * p + 128],
                            rhs=xt_sb[:, kk, 512 * ib:512 * ib + 512],
                            start=(kk == 0), stop=(kk == KK - 1))
                    for kk in range(KK):
                        nc.tensor.matmul(
                            psk[:], lhsT=wk_sb[:, kk, 128 * p:128 * p + 128],
                            rhs=xt_sb[:, kk, 512 * ib:512 * ib + 512],
                            start=(kk == 0), stop=(kk == KK - 1))
                    nc.vector.tensor_copy(qt[:, 512 * ib:512 * ib + 512], psq[:])
                    nc.vector.tensor_copy(kt[:, 512 * ib:512 * ib + 512], psk[:])

            # v_aug: 8 j-tiles [128, 4*65]; col 65h = 1.0, cols 65h+1.. = v_h
            vaug = []
            for jp in range(NJP):
                vt = acts.tile([128, HPC, 65], F32R, tag=f"vaug{jp}")
                vaug.append(vt)
                nc.gpsimd.dma_start(out=vt[:, :, 0], in_=onc_d[:].bitcast(F32R))
                psv = ps_big.tile([128, 256], F32, tag="big")
                for kk in range(KK):
                    nc.tensor.matmul(
                        psv[:], lhsT=xt_sb[:, kk, 128 * jp:128 * jp + 128],
                        rhs=wv_sb[:, kk, :],
                        start=(kk == 0), stop=(kk == KK - 1))
                nc.vector.tensor_copy(
                    vaug[jp][:, :, 1:65],
                    psv[:].rearrange("p (h d) -> p h d", h=HPC))

            # gates: per (h, ib) [65, 512] = sigmoid(Wg_aug^T x^T + bg)
            gT = [[None] * NB for _ in range(HPC)]
            for h in range(HPC):
                for ib in range(NB):
                    psg = ps_big.tile([65, 512], F32, tag="big")
                    for kk in range(KK):
                        nc.tensor.matmul(
                            psg[:], lhsT=wg_sb[:, kk, 65 * h:65 * h + 65],
                            rhs=xt_sb[:, kk, 512 * ib:512 * ib + 512],
                            start=(kk == 0), stop=(kk == KK - 1))
                    gt = gatep.tile([65, 512], F32, tag="gT")
                    nc.scalar.activation(gt[:], psg[:], AF.Sigmoid,
                                         bias=bg_sb[:, h:h + 1])
                    gT[h][ib] = gt

            # ---- phase 2+3: cross-step pipelined attention ----
            # Steps (ib, h); step s's qk/exp/mul interleave 1:1 with step
            # s-1's AV chain so AV operands are long since ready and the
            # po accumulation chain is spaced by independent matmuls.
            steps = [(ib, h) for ib in range(NB) for h in range(HPC)]
            og_by_ib = [[], []]
            prev = None

            def emit_tail(st):
                po = st["po"]
                r = small.tile([1, 512], F32R, tag="recip")
                nc.vector.reciprocal(r[:], po[0:1, :])
                pR = ps_m.tile([65, 512], F32, tag="misc")
                nc.tensor.matmul(pR[:], lhsT=ones65[:], rhs=r[:],
                                 start=True, stop=True)
                t1 = small.tile([65, 512], F32, tag="t1")
                nc.vector.tensor_mul(t1[:], po[:], gT[st["h"]][st["ib"]][:])
                og = outp.tile([65, 512], F32R, tag="outg")
                nc.vector.tensor_mul(og[:], t1[:], pR[:])
                og_by_ib[st["ib"]].append(og)
                if st["h"] == HPC - 1:
                    outg = og_by_ib[st["ib"]]
                    for half in range(2):
                        psy0 = ps_m.tile([128, 256], F32, tag="misc")
                        psy1 = ps_m.tile([128, 256], F32, tag="misc")
                        l0, l1 = 2 * half, 2 * half + 1
                        for hh in range(HPC):
                            nc.tensor.matmul(
                                psy0[:], lhsT=outg[hh][:, 128 * l0:128 * l0 + 128],
                                rhs=wo_sb[hh][:],
                                start=(hh == 0), stop=(hh == HPC - 1))
                            nc.tensor.matmul(
                                psy1[:], lhsT=outg[hh][:, 128 * l1:128 * l1 + 128],
                                rhs=wo_sb[hh][:],
                                start=(hh == 0), stop=(hh == HPC - 1))
                        for lp_, psy in ((l0, psy0), (l1, psy1)):
                            it = 4 * st["ib"] + lp_
                            yt = small.tile([128, 256], F32, tag="yt")
                            nc.vector.tensor_copy(yt[:], psy[:])
                            nc.gpsimd.dma_start(
                                out=y_d[128 * it:128 * it + 128, :], in_=yt[:])

            for s in range(len(steps) + 1):
                cur = None
                if s < len(steps):
                    ib, h = steps[s]
                    pair, off = h // 2, 64 * (h % 2)
                    bt = biasp.tile([128, NJP, 512], FP16, tag="bias")
                    bsrc = bias_d[h, ib].rearrange("p (j n) -> p j n", j=NJP)
                    nc.sync.dma_start(out=bt[:], in_=bsrc[:])
                    po_t = ps_o.tile([65, 512], F32, tag="po")
                    cur = {"ib": ib, "h": h, "po": po_t, "pts": []}
                for j in range(NJP):
                    if cur is not None:
                        ps = ps_big.tile([128, 512], F32, tag="big")
                        nc.tensor.matmul(
                            ps[:],
                            lhsT=kT[pair][off:off + 64, 128 * j:128 * j + 128],
                            rhs=qT[pair][off:off + 64, 512 * ib:512 * ib + 512],
                            start=True, stop=True)
                        if j % 2 == 0:
                            pe_pair = pexp.tile([128, 2, 512], FP16, tag="pexp")
                            cur["pe"] = pe_pair
                        nc.scalar.activation(cur["pe"][:, j % 2, :], ps[:], AF.Exp)
                        if j % 2 == 1:
                            ptp = pmul.tile([128, 2, 512], F32R, tag="pmul")
                            cur["pts"].append(ptp)
                            eng = nc.vector if (j // 2) != 3 else nc.gpsimd
                            eng.tensor_mul(ptp[:], cur["pe"][:],
                                           bt[:, j - 1:j + 1, :])
                    if prev is not None:
                        nc.tensor.matmul(
                            prev["po"][:],
                            lhsT=vaug[j][:, prev["h"], :],
                            rhs=prev["pts"][j // 2][:, j % 2, :],
                            start=(j == 0), stop=(j == NJP - 1))
                if prev is not None:
                    emit_tail(prev)
                prev = cur


_PROG = None


def _get_program():
    global _PROG
    if _PROG is None:
        _PROG = _build_program()
    return _PROG


def _prep_core_inputs(x, attn_bias, wq_s, wkv, wo, wg_s, bg, core):
    b, cp = core // 2, core % 2
    hs = HPC * cp
    f32 = np.float32

    xt = np.ascontiguousarray(
        x[b].T.reshape(KK, 128, N).transpose(1, 0, 2)).astype(f32, copy=False)

    A = attn_bias[b, hs:hs + HPC]                      # [4, i, j]
    bias_t = np.exp(np.ascontiguousarray(
        A.reshape(HPC, NB, 512, NJP, 128).transpose(0, 1, 4, 3, 2)
    ).reshape(HPC, NB, 128, NJP * 512).astype(f32, copy=False)
    ).astype(np.float16)

    def wtile(w):   # [256, 256] -> [128, KK, 256]
        return np.ascontiguousarray(
            w.reshape(KK, 128, 256).transpose(1, 0, 2)).astype(f32, copy=False)

    wq_t = wtile(wq_s[:, 256 * cp:256 * cp + 256] * SCALE)
    wk_t = wtile(wkv[:, :INNER][:, 256 * cp:256 * cp + 256])
    wv_t = wtile(wkv[:, INNER:][:, 256 * cp:256 * cp + 256])

    wg_aug = np.zeros((DIM, HPC * 65), f32)
    bg_aug = np.zeros((65, HPC), f32)
    wo_aug = np.zeros((HPC, 65, 256), f32)
    for h in range(HPC):
        g0 = 256 * cp + 64 * h
        wg_aug[:, 65 * h + 1:65 * h + 65] = wg_s[:, g0:g0 + 64]
        bg_aug[1:, h] = bg[g0:g0 + 64]
        wo_aug[h, 1:, :] = wo[g0:g0 + 64, :]
    wg_t = np.ascontiguousarray(
        wg_aug.reshape(KK, 128, HPC * 65).transpose(1, 0, 2))

    return {
        "xt": xt, "bias_t": bias_t, "wq": wq_t, "wk": wk_t, "wv": wv_t,
        "wg": wg_t, "bg": bg_aug, "wo": wo_aug,
        "ones65": np.ones((1, 65), f32),
        "ones128": np.ones((128, HPC), f32),
    }


_LAST_RESULTS = None


def kernel(x, attn_bias, Wq, Wkv, Wo, bo, Wg, bg, _trace=False, **_trace_kw):
    global _LAST_RESULTS
    x = np.asarray(x, np.float32)
    attn_bias = np.asarray(attn_bias, np.float32)
    Wq = np.asarray(Wq, np.float32)
    Wkv = np.asarray(Wkv, np.float32)
    Wo = np.asarray(Wo, np.float32)
    bo = np.asarray(bo, np.float32)
    Wg = np.asarray(Wg, np.float32)
    bg = np.asarray(bg, np.float32)

    nc = _get_program()
    in_maps = [_prep_core_inputs(x, attn_bias, Wq, Wkv, Wo, Wg, bg, c)
               for c in range(NCORES)]
    res = run_bass_kernel_spmd(nc, in_maps, list(range(NCORES)),
                               trace=_trace, **_trace_kw)
    _LAST_RESULTS = res

    y = np.empty((B, N, DIM), np.float32)
    for b in range(B):
        y[b] = res.results[2 * b]["y"] + res.results[2 * b + 1]["y"] + bo
    return y



# revision 4
# speedup vs baseline: 1.3787x; 1.3787x over previous
"""Trainium2 Bass kernel for nn_Attention_46420006535531 (v2).

Gated multi-head attention with additive attention bias:
    q = x@Wq, (k, v) = split(x@Wkv), heads=8, dim_head=64
    attn = softmax(q*k^T*scale + bias); out = attn@v
    out = (out * sigmoid(x@Wg + bg)) @ Wo + bo

Sharding: 8 cores; core c handles batch b=c//2 and the 4 heads
4*(c%2)..4*(c%2)+3.  Each core computes a partial y (its heads' slice
of Wo rows); the host sums the two partials per batch and adds bo.

v2 layout notes (all on-core data transposed, fp16 pipeline):
 - S^T[j,i] per head in [128,512] tiles; two tiles share one
   [128,1024] PSUM buffer so ACT exps 1024 elements per instruction.
 - bias enters as exp(bias)^T fp16 (host-prepped); attention weights
   are exp(S)*exp(bias) via fp16 DVE/Pool muls (2x DVE mode).
 - softmax denominator rides row 0 of the AV output (ones column in
   the augmented v tile); reciprocal is broadcast to 64 partitions by
   a [1,64]x[1,512] matmul of 0.5-constants (the 0.5 folds the tanh
   gate identity sigmoid(z) = 0.5 + 0.5*tanh(z/2), so gates use Tanh
   and stay in the same ACT table as Exp - no table reloads).
 - gates/out-proj pack two heads along 128 partitions (full PE rows).
 - y is DMAd straight out of PSUM; all DMA goes through sync/HWDGE.
"""
import sys
import numpy as np

for _p in ("/opt/trn_rl_repo",):
    if _p not in sys.path:
        sys.path.insert(0, _p)

import concourse.bass as bass
import concourse.bacc as bacc
import concourse.tile as tile
from concourse import mybir
from concourse.bass_utils import run_bass_kernel_spmd

B, N, DIM = 4, 1024, 256
HEADS, DIM_HEAD, INNER = 8, 64, 512
HPC = 4                      # heads per core
NCORES = 8
SCALE = DIM_HEAD ** -0.5     # folded into Wq on the host

F32 = mybir.dt.float32
FP16 = mybir.dt.float16
AF = mybir.ActivationFunctionType
ALU = mybir.AluOpType

NB = N // 512                # 2 i-blocks of 512
NJP = N // 128               # 8 j partition tiles
KK = DIM // 128              # 2 k-tiles for the projections


def _build_program(reps=1, loop_iters=0):
    nc = bacc.Bacc(None, target_bir_lowering=False)

    # ---- DRAM I/O (per core) ----
    xt_d = nc.dram_tensor("xt", [128, KK, N], FP16, kind="ExternalInput")
    bias_d = nc.dram_tensor("bias_t", [HPC, NB, 128, NJP * 512], FP16,
                            kind="ExternalInput")
    wq_d = nc.dram_tensor("wq", [128, KK, 256], FP16, kind="ExternalInput")
    wk_d = nc.dram_tensor("wk", [128, KK, 256], FP16, kind="ExternalInput")
    wv_d = nc.dram_tensor("wv", [128, KK, 256], FP16, kind="ExternalInput")
    wg_d = nc.dram_tensor("wg", [128, KK, 256], FP16, kind="ExternalInput")
    bgh_d = nc.dram_tensor("bgh", [128, 2], F32, kind="ExternalInput")
    wo_d = nc.dram_tensor("wo", [2, 128, 256], FP16, kind="ExternalInput")
    hv_d = nc.dram_tensor("halves64", [1, 64], FP16, kind="ExternalInput")
    y_d = nc.dram_tensor("y", [N, 256], FP16, kind="ExternalOutput")

    with tile.TileContext(nc) as tc:
        import contextlib
        with contextlib.ExitStack() as ctx:
            const = ctx.enter_context(tc.tile_pool(name="const", bufs=1))
            acts = ctx.enter_context(tc.tile_pool(name="acts", bufs=2))
            biasp = ctx.enter_context(tc.tile_pool(name="biasp", bufs=3))
            pexp = ctx.enter_context(tc.tile_pool(name="pexp", bufs=6))
            pmul = ctx.enter_context(tc.tile_pool(name="pmul", bufs=8))
            small = ctx.enter_context(tc.tile_pool(name="small", bufs=4))
            ps_big = ctx.enter_context(tc.tile_pool(name="ps_big", bufs=2, space="PSUM"))
            ps_o = ctx.enter_context(tc.tile_pool(name="ps_o", bufs=2, space="PSUM"))
            ps_m = ctx.enter_context(tc.tile_pool(name="ps_m", bufs=2, space="PSUM"))

            # ---- constants / weights into SBUF (once) ----
            halves = const.tile([1, 64], FP16, tag="halves64")
            nc.sync.dma_start(out=halves[:], in_=hv_d[:])
            bgh_sb = const.tile([128, 2], F32, tag="bgh")
            nc.sync.dma_start(out=bgh_sb[:], in_=bgh_d[:])
            wq_sb = const.tile([128, KK, 256], FP16, tag="wq")
            nc.sync.dma_start(out=wq_sb[:], in_=wq_d[:])
            wk_sb = const.tile([128, KK, 256], FP16, tag="wk")
            nc.sync.dma_start(out=wk_sb[:], in_=wk_d[:])
            wv_sb = const.tile([128, KK, 256], FP16, tag="wv")
            nc.sync.dma_start(out=wv_sb[:], in_=wv_d[:])
            wg_sb = const.tile([128, KK, 256], FP16, tag="wg")
            nc.sync.dma_start(out=wg_sb[:], in_=wg_d[:])
            wo_sb = []
            for p in range(2):
                t = const.tile([128, 256], FP16, tag=f"wo{p}")
                nc.sync.dma_start(out=t[:], in_=wo_d[p])
                wo_sb.append(t)
            xt_sb = const.tile([128, KK, N], FP16, tag="xt")
            nc.sync.dma_start(out=xt_sb[:], in_=xt_d[:])

            env = dict(locals())
            lp = nc.allow_low_precision(reason="fp16 attention pipeline")
            lp.__enter__()

            if loop_iters:
                with tc.For_i(0, loop_iters, 1):
                    _emit_body(nc, tc, env)
            else:
                for _rep in range(reps):
                    _emit_body(nc, tc, env)

            lp.__exit__(None, None, None)

    nc.compile()
    return nc


def _emit_body(nc, tc, env):
    acts = env["acts"]; biasp = env["biasp"]
    pexp = env["pexp"]; pmul = env["pmul"]; small = env["small"]
    ps_big = env["ps_big"]; ps_o = env["ps_o"]; ps_m = env["ps_m"]
    halves = env["halves"]; bgh_sb = env["bgh_sb"]
    wq_sb = env["wq_sb"]; wk_sb = env["wk_sb"]; wv_sb = env["wv_sb"]
    wg_sb = env["wg_sb"]; wo_sb = env["wo_sb"]; xt_sb = env["xt_sb"]
    bias_d = env["bias_d"]; y_d = env["y_d"]

    # ---- phase 1: projections ----
    # qT / kT per head-pair p: [128, N] fp16 (rows = 2 heads x 64 dims)
    qT, kT = [], []
    for p in range(2):
        qt = acts.tile([128, N], FP16, tag=f"qT{p}")
        kt = acts.tile([128, N], FP16, tag=f"kT{p}")
        qT.append(qt)
        kT.append(kt)
        for ib in range(NB):
            psqk = ps_big.tile([128, 1024], F32, tag="big")
            for kk in range(KK):
                nc.tensor.matmul(
                    psqk[:, 0:512], lhsT=wq_sb[:, kk, 128 * p:128 * p + 128],
                    rhs=xt_sb[:, kk, 512 * ib:512 * ib + 512],
                    start=(kk == 0), stop=(kk == KK - 1))
            for kk in range(KK):
                nc.tensor.matmul(
                    psqk[:, 512:1024], lhsT=wk_sb[:, kk, 128 * p:128 * p + 128],
                    rhs=xt_sb[:, kk, 512 * ib:512 * ib + 512],
                    start=(kk == 0), stop=(kk == KK - 1))
            nc.vector.tensor_copy(qt[:, 512 * ib:512 * ib + 512], psqk[:, 0:512])
            nc.vector.tensor_copy(kt[:, 512 * ib:512 * ib + 512], psqk[:, 512:1024])

    # gates: thT[pair][ib] = tanh(0.5*(x@Wg_pair) + 0.5*bg_pair)  [128, 512]
    thT = [[None] * NB for _ in range(2)]
    for p in range(2):
        for ib in range(NB):
            psg = ps_m.tile([128, 512], F32, tag="misc")
            for kk in range(KK):
                nc.tensor.matmul(
                    psg[:], lhsT=wg_sb[:, kk, 128 * p:128 * p + 128],
                    rhs=xt_sb[:, kk, 512 * ib:512 * ib + 512],
                    start=(kk == 0), stop=(kk == KK - 1))
            gt = acts.tile([128, 512], FP16, tag=f"thT{p}{ib}")
            nc.scalar.activation(gt[:], psg[:], AF.Tanh,
                                 bias=bgh_sb[:, p:p + 1], scale=0.5)
            thT[p][ib] = gt

    # v_aug: 8 j-tiles [128, 4, 65] fp16; col 64 of each head = 1.0
    vaug = []
    for jp in range(NJP):
        vt = acts.tile([128, HPC, 65], FP16, tag=f"vaug{jp}")
        vaug.append(vt)
        nc.gpsimd.memset(vt[:, :, 64], 1.0)
        psv = ps_m.tile([128, 256], F32, tag="misc")
        for kk in range(KK):
            nc.tensor.matmul(
                psv[:], lhsT=xt_sb[:, kk, 128 * jp:128 * jp + 128],
                rhs=wv_sb[:, kk, :],
                start=(kk == 0), stop=(kk == KK - 1))
        nc.vector.tensor_copy(
            vt[:, :, 0:64],
            psv[:].rearrange("p (h d) -> p h d", h=HPC))

    # ---- phase 2: cross-step pipelined attention ----
    # Steps (ib, h); step s's qk/exp/mul interleave 1:1 with step s-1's
    # AV chain.
    steps = [(ib, h) for ib in range(NB) for h in range(HPC)]
    og_by_ib = [[None] * HPC for _ in range(NB)]
    og_tiles = {}
    prev = None
    mul_ctr = [0]

    def emit_tail(st):
        ib, h = st["ib"], st["h"]
        po = st["po"]
        pair, half = h // 2, h % 2
        r = small.tile([1, 512], FP16, tag="recip")
        nc.vector.reciprocal(r[:], po[64:65, :])
        pR = ps_m.tile([64, 512], F32, tag="misc")
        nc.tensor.matmul(pR[:], lhsT=halves[:], rhs=r[:],
                         start=True, stop=True)
        t1 = small.tile([64, 512], FP16, tag="t1")
        nc.vector.scalar_tensor_tensor(
            t1[:], thT[pair][ib][64 * half:64 * half + 64, :], 1.0,
            po[0:64, :], ALU.add, ALU.mult)
        key = (ib, pair)
        if key not in og_tiles:
            og = acts.tile([128, 512], FP16, tag=f"og{ib}{pair}")
            og_tiles[key] = og
        og = og_tiles[key]
        nc.vector.tensor_mul(og[64 * half:64 * half + 64, :], t1[:], pR[:])
        og_by_ib[ib][h] = og
        if h == HPC - 1:
            for ic in range(4):
                psy = ps_m.tile([128, 256], F32, tag="misc")
                for pr in range(2):
                    nc.tensor.matmul(
                        psy[:],
                        lhsT=og_tiles[(ib, pr)][:, 128 * ic:128 * ic + 128],
                        rhs=wo_sb[pr][:],
                        start=(pr == 0), stop=(pr == 1))
                it = 4 * ib + ic
                yt = small.tile([128, 256], FP16, tag="yt")
                nc.vector.tensor_copy(yt[:], psy[:])
                nc.sync.dma_start(out=y_d[128 * it:128 * it + 128, :],
                                  in_=yt[:])
            for pr in range(2):
                del og_tiles[(ib, pr)]

    nsteps = len(steps)
    for s in range(nsteps + 1):
        cur = None
        if s < nsteps:
            ib, h = steps[s]
            pair, off = h // 2, 64 * (h % 2)
            bt = biasp.tile([128, NJP, 512], FP16, tag="bias")
            bsrc = bias_d[h, ib].rearrange("p (j n) -> p j n", j=NJP)
            nc.sync.dma_start(out=bt[:], in_=bsrc[:])
            po_t = ps_o.tile([65, 512], F32, tag="po")
            cur = {"ib": ib, "h": h, "po": po_t, "pts": []}
        for j in range(NJP):
            if cur is not None:
                if j % 2 == 0:
                    ps_t = ps_big.tile([128, 1024], F32, tag="big")
                    cur["ps"] = ps_t
                nc.tensor.matmul(
                    cur["ps"][:, 512 * (j % 2):512 * (j % 2) + 512],
                    lhsT=kT[pair][off:off + 64, 128 * j:128 * j + 128],
                    rhs=qT[pair][off:off + 64, 512 * ib:512 * ib + 512],
                    start=True, stop=True)
                if j % 2 == 1:
                    pe = pexp.tile([128, 2, 512], FP16, tag="pexp")
                    nc.scalar.activation(
                        pe[:].rearrange("p a n -> p (a n)"),
                        cur["ps"][:], AF.Exp)
                    ptp = pmul.tile([128, 2, 512], FP16, tag="pmul")
                    cur["pts"].append(ptp)
                    eng = nc.vector if (mul_ctr[0] % 16) == 0 else nc.gpsimd
                    mul_ctr[0] += 1
                    eng.tensor_mul(ptp[:], pe[:], bt[:, j - 1:j + 1, :])
            if prev is not None:
                nc.tensor.matmul(
                    prev["po"][:],
                    lhsT=vaug[j][:, prev["h"], :],
                    rhs=prev["pts"][j // 2][:, j % 2, :],
                    start=(j == 0), stop=(j == NJP - 1))
        if prev is not None:
            emit_tail(prev)
        prev = cur


_PROG = None


def _get_program():
    global _PROG
    if _PROG is None:
        _PROG = _build_program()
    return _PROG


def _prep_core_inputs(x, attn_bias, Wq, Wkv, Wo, Wg, bg, core):
    b, cp = core // 2, core % 2
    f16 = np.float16
    f32 = np.float32

    xt = np.ascontiguousarray(
        x[b].T.reshape(KK, 128, N).transpose(1, 0, 2)).astype(f16)

    hs = HPC * cp
    A = attn_bias[b, hs:hs + HPC]                      # [4, i, j]
    bias_t = np.exp(np.ascontiguousarray(
        A.reshape(HPC, NB, 512, NJP, 128).transpose(0, 1, 4, 3, 2)
    ).reshape(HPC, NB, 128, NJP * 512).astype(f32, copy=False)
    ).astype(f16)

    def wtile(w):   # [256, 256] -> [128, KK, 256] fp16
        return np.ascontiguousarray(
            w.reshape(KK, 128, 256).transpose(1, 0, 2)).astype(f16)

    wq_t = wtile(Wq[:, 256 * cp:256 * cp + 256] * SCALE)
    wk_t = wtile(Wkv[:, :INNER][:, 256 * cp:256 * cp + 256])
    wv_t = wtile(Wkv[:, INNER:][:, 256 * cp:256 * cp + 256])
    wg_t = wtile(Wg[:, 256 * cp:256 * cp + 256])

    g0 = 256 * cp
    bgh = np.zeros((128, 2), f32)
    for p in range(2):
        bgh[:, p] = 0.5 * bg[g0 + 128 * p:g0 + 128 * p + 128]
    wo_t = np.ascontiguousarray(
        Wo[g0:g0 + 256, :].reshape(2, 128, 256)).astype(f16)

    return {
        "xt": xt, "bias_t": bias_t, "wq": wq_t, "wk": wk_t, "wv": wv_t,
        "wg": wg_t, "bgh": bgh, "wo": wo_t,
        "halves64": np.full((1, 64), 0.5, f16),
    }


_LAST_RESULTS = None


def kernel(x, attn_bias, Wq, Wkv, Wo, bo, Wg, bg, _trace=False, **_trace_kw):
    global _LAST_RESULTS
    x = np.asarray(x, np.float32)
    attn_bias = np.asarray(attn_bias, np.float32)
    Wq = np.asarray(Wq, np.float32)
    Wkv = np.asarray(Wkv, np.float32)
    Wo = np.asarray(Wo, np.float32)
    bo = np.asarray(bo, np.float32)
    Wg = np.asarray(Wg, np.float32)
    bg = np.asarray(bg, np.float32)

    nc = _get_program()
    in_maps = [_prep_core_inputs(x, attn_bias, Wq, Wkv, Wo, Wg, bg, c)
               for c in range(NCORES)]
    res = run_bass_kernel_spmd(nc, in_maps, list(range(NCORES)),
                               trace=_trace, **_trace_kw)
    _LAST_RESULTS = res

    y = np.empty((B, N, DIM), np.float32)
    for b in range(B):
        y[b] = (res.results[2 * b]["y"].astype(np.float32)
                + res.results[2 * b + 1]["y"].astype(np.float32) + bo)
    return y
